# revision 44
# baseline (speedup 1.0000x reference)
"""CapsuleConv2d (k-means routing, 3 iters) Trainium2 Bass kernel.

Problem (hardcoded): x [2,128,32,32] f32, weight [16,16,16,3,3] f32
(w[o,l,m,i,j]), stride 1, pad 1, G=8 groups of M=16 in-channels,
N_in = G*KH*KW = 72 votes, O=16 out-capsules of L=16.
Output [2, 256, 32, 32] f32.

Sharding: data-parallel over (b, oh): 64 rows -> 8 cores x 8 rows.
Each core processes 2 chunks of 128 positions (4 oh-rows x 32 ow).

Per-chunk pipeline (single NeuronCore):
  PE:  priors u[p,(n,l,o)] via 72 fp32r matmuls [K=16(m), M=128(p),
       N=256(l,o)] + 9 accumulating K=128 matmuls for v0 = sum_n u.
  ACT: PSUM->SBUF u drains (cast bf16), exp, rsqrt/sqrt via exp/ln
       (single act-table set: Exp/Ln/Copy/Identity pinned to one set).
  Routing elementwise work in layout [p partitions, (n, l, o) free] is
  SPLIT by n between Pool (n 0:16, tensor_add/mul at 0.42 Q7 efficiency
  ~2.0ns/elem) and DVE (n 16:72, bf16 2x mode 0.52ns/elem), per iter:
       z1 = u*bcast(vn) -> l-tree -> logits(bf16) -> e = exp (ACT, split
       3 ways so each engine's z2 gates only on its own logits)
       z2 = u*bcast(e) -> n-fold per engine -> v_u merge (Pool absorbs
       the DVE fold carry so the merge is 2 DVE adds)
  prep is split: rn chain right after each main, the vn multiply only
  after the OTHER chunk's main (DVE queue head never parks on ACT).
  final: squash fused with softmax denom: out = v_u*||v_u||/(S^2+||v_u||^2),
       PE-transpose [p,(o,l)] -> [(o,l),p], channel-major output DMA.
"""
from contextlib import ExitStack

import numpy as np

B, CIN, H, W = 2, 128, 32, 32
G, M, O, L = 8, 16, 16, 16
NTAP, NIN = 9, 72
COUT = O * L
NCORES = 8
ROWS_PER_CORE = 8  # (b, oh) rows per core
CHUNK_ROWS = 4
NCHUNK = ROWS_PER_CORE // CHUNK_ROWS
P = 128
NPOOL = 16               # n-blocks routed on the Pool engine (n 0:16)
NDVE = NIN - NPOOL       # 56, on DVE (n 16:72)
NA = 20                  # DVE section A (n 16:36; with Pool: exp half 1)
NB = NDVE - NA           # DVE section B (n 36:72, 36 blocks; exp half 2)
BL = 256                 # one n-block = L*O elems


def _build_bass():
    import concourse.tile as tile
    from concourse import bacc, masks, mybir

    # The act-table pass greedily picks the first set containing each
    # function, ping-ponging exp_and_others <-> natural_log (2.7us/load).
    # Strip Exp/Ln from every set except the combined one so all our ACT
    # work (Exp, Ln, Copy, Identity) lives in a single table set.
    if not getattr(bacc, "_capsule_act_tables_patched", False):
        _orig_gat = bacc.get_activation_tables

        def _gat(arch):
            t = dict(_orig_gat(arch))
            for name, fns in t.items():
                if name != "natural_log_exp_and_others":
                    t[name] = {f for f in fns
                               if f.name not in ("Exp", "Ln", "Copy",
                                                 "Identity")}
            return t

        bacc.get_activation_tables = _gat
        bacc._capsule_act_tables_patched = True

    fp32 = mybir.dt.float32
    f32r = mybir.dt.float32r
    bf16 = mybir.dt.bfloat16
    AX = mybir.AxisListType
    AF = mybir.ActivationFunctionType
    ALU = mybir.AluOpType

    nc = bacc.Bacc("TRN2", target_bir_lowering=False, debug=False)
    # host-pretransposed, pre-padded slab: xs[m, g, h(10), w(34)]
    xs_d = nc.declare_dram_parameter("xs", [M, G, 10, 34], f32r,
                                     isOutput=False)
    # same slab, channel-major: xs2[c=(g,m), h(10), w(34)]
    xs2_d = nc.declare_dram_parameter("xs2", [CIN, 10, 34], f32r,
                                      isOutput=False)
    # host-pretransposed weights: wr[m, (tap, l, o)] = w[o, l, m, i, j]
    w_d = nc.declare_dram_parameter("wgt", [M, NTAP * 256], fp32, isOutput=False)
    out_d = nc.declare_dram_parameter("out", [COUT, ROWS_PER_CORE, W], fp32,
                                      isOutput=True)

    with tile.TileContext(nc) as tc, ExitStack() as ctx:
        const_pool = ctx.enter_context(tc.tile_pool(name="const", bufs=1))
        upool = ctx.enter_context(tc.tile_pool(name="u", bufs=2))
        zpool = ctx.enter_context(tc.tile_pool(name="z", bufs=1))
        small = ctx.enter_context(tc.tile_pool(name="small", bufs=2))
        psum = ctx.enter_context(tc.tile_pool(name="ps", bufs=4, space="PSUM"))
        tpsum = ctx.enter_context(tc.tile_pool(name="tps", bufs=2, space="PSUM"))
        vpsum = ctx.enter_context(tc.tile_pool(name="vps", bufs=2, space="PSUM"))

        def dadd(out, a, b):
            nc.vector.tensor_add(out, a, b)

        def padd(out, a, b):
            nc.gpsimd.tensor_add(out, a, b)

        def pmul(out, a, b):
            nc.gpsimd.tensor_mul(out, a, b)

        # ---- constants (once per core) ----
        # x slab first: it gates the patch pipeline (weights only gate the
        # matmuls, which start later anyway)
        slab_f = const_pool.tile([M, G * 10 * 34], f32r)
        nc.sync.dma_start(out=slab_f[:],
                          in_=xs_d[:].rearrange("m g h w -> m (g h w)"))
        slab = slab_f[:].rearrange("m (g h w) -> m g h w", g=G, h=10)

        # channel-major slab for the K=128 v0 matmuls; f32r so the windowed
        # patch2 copies go over plain (cast-free) SP-queue DMAs
        slab2_f = const_pool.tile([CIN, 10 * 34], f32r)
        nc.sync.dma_start(out=slab2_f[:],
                          in_=xs2_d[:].rearrange("c h w -> c (h w)"))
        slab2 = slab2_f[:].rearrange("c (h w) -> c h w", h=10)

        # weights replicated over g: wr2[(g,m), (t,l,o)]; wr_r = g=0 slice.
        wr2_f = const_pool.tile([CIN, NTAP * 256], fp32)
        for g in range(G):
            eng = nc.sync if g % 2 == 0 else nc.scalar
            eng.dma_start(out=wr2_f[g * M:(g + 1) * M, :], in_=w_d[:])
        wr2 = const_pool.tile([CIN, NTAP * 256], f32r)
        nc.vector.tensor_copy(wr2[:], wr2_f[:])
        wr_r = wr2[0:M, :]

        ident = const_pool.tile([128, 128], fp32)
        masks.make_identity(nc, ident[:])

        # PE warm-up: ~4us of back-to-back dummy matmuls during the initial
        # DMA wait releases the HAM clock throttle before the real matmuls.
        warm = const_pool.tile([128, 64], bf16)
        nc.vector.memset(warm[:], 0.0)
        wps = tpsum.tile([64, 64], fp32, tag="tp")
        for _ in range(40):
            nc.tensor.matmul(wps[:], warm[:, 0:64], warm[:], start=True,
                             stop=True, tile_position=(0, 0))

        ppool = ctx.enter_context(tc.tile_pool(name="patch", bufs=3))
        p2pool = ctx.enter_context(tc.tile_pool(name="patch2", bufs=3))

        def priors(k):
            # ---- priors on PE (fp32r: full-rate, near-fp32 precision) ----
            # Chunk 0 (nothing to overlap with): Pool does patch2 + v0 copy
            # + the first two tap patches (everything its own z1P needs),
            # then goes idle so routing can start; ACT does the remaining
            # patches and all drains n>=16; DVE drains n 0:16 (Pool's
            # range) so z1P unblocks early. Chunk 1 (under chunk-0
            # routing): patch2 rides the idle SP DMA queue, patches and
            # drains go to ACT.
            u_sb = upool.tile([P, NIN * 256], bf16, tag="u")
            v0_ps = vpsum.tile([P, 256], fp32, tag="v0ps")
            v_u = small.tile([P, 256], fp32, tag="vu")
            # all v0 matmuls first: v0 completes early so the first prep
            # chain overlaps the u fill instead of following it
            for t in range(NTAP):
                i, j = divmod(t, 3)
                # channel-major window for the K=128 v0 matmul on the idle
                # SP DMA queue (3-dim AP fits the DMA cap)
                patch2 = p2pool.tile([CIN, P], f32r, tag="patch2")
                nc.sync.dma_start(
                    out=patch2[:].rearrange("c (h w) -> c h w",
                                            h=CHUNK_ROWS),
                    in_=slab2[:, 4 * k + i: 4 * k + i + CHUNK_ROWS,
                              j:j + W])
                nc.tensor.matmul(v0_ps[:], patch2[:],
                                 wr2[:, t * 256:(t + 1) * 256],
                                 start=(t == 0), stop=(t == NTAP - 1),
                                 tile_position=(0, 0), skip_group_check=True)
            nc.scalar.copy(v_u[:], v0_ps[:])
            for t in range(NTAP):
                i, j = divmod(t, 3)
                # contiguous tap window: patch[m, (g, poh, ow)]
                patch = ppool.tile([M, G * P], f32r, tag="patch")
                # chunk-0 fill is patch-cadence-bound: alternate Pool/ACT
                peng = (nc.gpsimd.tensor_copy if (k == 0 and t % 2 == 0)
                        else nc.scalar.copy)
                peng(
                    patch[:].rearrange("m (g h w) -> m g h w", g=G,
                                       h=CHUNK_ROWS),
                    slab[:, :, 4 * k + i: 4 * k + i + CHUNK_ROWS, j:j + W])
                rhs = wr_r[:, t * 256:(t + 1) * 256]
                for g in range(G):
                    n = t * G + g
                    lhsT = patch[:, g * P:(g + 1) * P]
                    if n % 2 == 0:
                        ups = psum.tile([P, 512], fp32, tag="ups")
                    nc.tensor.matmul(ups[:, (n % 2) * 256:(n % 2 + 1) * 256],
                                     lhsT, rhs, start=True, stop=True,
                                     tile_position=(0, 0))
                    if n % 2 == 1:
                        # drain two adjacent u blocks with one copy; during
                        # the chunk-0 fill DVE is idle, so share the load
                        dst = u_sb[:, (n - 1) * 256:(n + 1) * 256]
                        if k == 0 and (n // 2) % 2 == 0:
                            nc.vector.tensor_copy(dst, ups[:])
                        else:
                            nc.scalar.copy(dst, ups[:])
            S = small.tile([P, O], fp32, tag="S")
            return {"u_sb": u_sb, "v_u": v_u, "S": S, "vn": None}

        def prep_a(st, hi=True):
            # rn = rsqrt(max(sum_l v_u^2, eps)); rsqrt = exp(-ln/2)
            # hi=False for the post-priors preps: a high-priority Ln there
            # jumps to the ACT queue front and head-blocks the fill drains
            # behind it while waiting on w2
            v_u = st["v_u"]
            sq = small.tile([P, 256], fp32, tag="sq")
            nc.vector.tensor_mul(sq[:], v_u[:], v_u[:])
            w2 = small.tile([P, O], fp32, tag="w2")
            nc.vector.reduce_sum(
                w2[:], sq[:].rearrange("p (l o) -> p o l", l=L), axis=AX.X)
            nc.vector.tensor_scalar_max(w2[:], w2[:], 1e-24)
            lg = small.tile([P, O], fp32, tag="lg")
            rn = small.tile([P, O], fp32, tag="rn")
            if hi:
                with tc.high_priority():
                    nc.scalar.activation(lg[:], w2[:], AF.Ln)
                    nc.scalar.activation(rn[:], lg[:], AF.Exp, scale=-0.5)
            else:
                nc.scalar.activation(lg[:], w2[:], AF.Ln)
                nc.scalar.activation(rn[:], lg[:], AF.Exp, scale=-0.5)
            st["rn"] = rn

        def prep_b(st):
            # vn = v_u * bcast(rn); emitted AFTER the other chunk's main so
            # the DVE queue head never blocks on the ACT Ln/Exp round-trip
            v_u, rn = st["v_u"], st["rn"]
            vn = small.tile([P, 256], bf16, tag="vn")
            nc.vector.tensor_mul(
                vn[:].rearrange("p (l o) -> p l o", l=L),
                v_u[:].rearrange("p (l o) -> p l o", l=L),
                rn[:].unsqueeze(1).broadcast_to([P, L, O]))
            st["vn"] = vn

        def main(st, it):
            u_sb, vn, S = st["u_sb"], st["vn"], st["S"]
            # one tile: z region [0:18432] + tree scratch
            # scratch layout (bf16 elems):
            #   trD  [18432:25600]  7168  DVE l-tree + n-fold ping-pong
            #   trP  [25600:28160]  2560  Pool l-tree + n-fold ping-pong
            zt = zpool.tile([P, NIN * BL + 9728], bf16, tag="z")
            z = zt[:, 0:NIN * BL]
            trD = zt[:, NIN * BL:NIN * BL + 7168]
            trP = zt[:, NIN * BL + 7168:NIN * BL + 9728]

            # bf16 logits: |logits| <= ~5.1 so bf16 rounding (<=0.02 abs)
            # perturbs exp by <2%; buys 2x mode for the final l-tree adds
            logits = small.tile([P, NIN * O], bf16, tag="logits")
            e = small.tile([P, NIN * O], bf16, tag="e")

            vnb = lambda nn: (vn[:].rearrange("p (l o) -> p l o", l=L)
                              .unsqueeze(1).broadcast_to([P, nn, L, O]))

            def zview(ap, nn, lw=L):
                return ap.rearrange("p (n l o) -> p n l o", n=nn, l=lw)

            def ltree(eng_add, zsec, nn, tr, lgsec):
                # zsec: z-slice [p, nn*256]; tr: scratch >= nn*128
                zv = zview(zsec, nn)
                t1 = tr[:, 0:nn * 128].rearrange(
                    "p (n l o) -> p n l o", n=nn, l=8)
                eng_add(t1, zv[:, :, 0:8, :], zv[:, :, 8:16, :])
                t2 = zview(zsec[:, 0:nn * 64], nn, 4)
                eng_add(t2, t1[:, :, 0:4, :], t1[:, :, 4:8, :])
                t3 = tr[:, 0:nn * 32].rearrange(
                    "p (n l o) -> p n l o", n=nn, l=2)
                eng_add(t3, t2[:, :, 0:2, :], t2[:, :, 2:4, :])
                eng_add(lgsec.rearrange("p (n o) -> p n o", n=nn),
                        t3[:, :, 0, :], t3[:, :, 1, :])

            # Section layout: Pool owns n 0:16 (zP), DVE owns n 16:36 (zA)
            # and 36:72 (zB). Three exp splits so each engine's z2 gates
            # only on its own logits (no cross-engine phase lag).
            zP = z[:, 0:NPOOL * BL]
            zA = z[:, NPOOL * BL:(NPOOL + NA) * BL]
            zB = z[:, (NPOOL + NA) * BL:NIN * BL]
            uP = zview(u_sb[:, 0:NPOOL * BL], NPOOL)
            uA = zview(u_sb[:, NPOOL * BL:(NPOOL + NA) * BL], NA)
            uB = zview(u_sb[:, (NPOOL + NA) * BL:NIN * BL], NB)

            # --- z1 = u * bcast_n(vn), l-tree -> logits ---
            pmul(zview(zP, NPOOL), uP, vnb(NPOOL))
            ltree(padd, zP, NPOOL, trP, logits[:, 0:NPOOL * O])
            nc.vector.tensor_mul(zview(zA, NA), uA, vnb(NA))
            ltree(dadd, zA, NA, trD, logits[:, NPOOL * O:(NPOOL + NA) * O])
            # exp order by expected logits readiness: A (~5us), P (~14), B
            with tc.high_priority():
                nc.scalar.activation(
                    e[:, NPOOL * O:(NPOOL + NA) * O],
                    logits[:, NPOOL * O:(NPOOL + NA) * O], AF.Exp)
            nc.vector.tensor_mul(zview(zB, NB), uB, vnb(NB))
            ltree(dadd, zB, NB, trD[:, NA * 128:NA * 128 + NB * 128],
                  logits[:, (NPOOL + NA) * O:NIN * O])
            with tc.high_priority():
                nc.scalar.activation(e[:, 0:NPOOL * O],
                                     logits[:, 0:NPOOL * O], AF.Exp)
                nc.scalar.activation(e[:, (NPOOL + NA) * O:NIN * O],
                                     logits[:, (NPOOL + NA) * O:NIN * O],
                                     AF.Exp)

            def ebc(lo, nn):
                return (e[:, lo * O:(lo + nn) * O]
                        .rearrange("p (n o) -> p n o", n=nn)
                        .unsqueeze(2).broadcast_to([P, nn, L, O]))

            # --- z2 = u * bcast_l(e) ---
            pmul(zview(zP, NPOOL), uP, ebc(0, NPOOL))
            nc.vector.tensor_mul(zview(zA, NA), uA, ebc(NPOOL, NA))
            nc.vector.tensor_mul(zview(zB, NB), uB, ebc(NPOOL + NA, NB))

            # --- n-fold: Pool over its 16 blocks, DVE over its 56 ---
            # Pool: 16 -> 8 -> 4 -> 2 -> 1 (clean), then absorb the DVE
            # carry c2 (Pool leads in phase; saves DVE merge adds)
            padd(trP[:, 0:8 * BL], zP[:, 0:8 * BL], zP[:, 8 * BL:16 * BL])
            padd(zP[:, 0:4 * BL], trP[:, 0:4 * BL], trP[:, 4 * BL:8 * BL])
            padd(trP[:, 8 * BL:10 * BL], zP[:, 0:2 * BL],
                 zP[:, 2 * BL:4 * BL])
            vPa = small.tile([P, 256], fp32, tag="vPa")
            padd(vPa[:], trP[:, 8 * BL:9 * BL], trP[:, 9 * BL:10 * BL])

            # DVE: 56 -> 28 -> 14 -> 7 -> 3(+c2) -> 1(+c3); blocks start
            # at offset 16*BL (zD = z[16BL:72BL])
            o0 = NPOOL * BL
            dadd(trD[:, 0:28 * BL], z[:, o0:o0 + 28 * BL],
                 z[:, o0 + 28 * BL:o0 + 56 * BL])
            dadd(z[:, o0:o0 + 14 * BL], trD[:, 0:14 * BL],
                 trD[:, 14 * BL:28 * BL])
            dadd(trD[:, 0:7 * BL], z[:, o0:o0 + 7 * BL],
                 z[:, o0 + 7 * BL:o0 + 14 * BL])
            dadd(z[:, o0:o0 + 3 * BL], trD[:, 0:3 * BL], trD[:, 3 * BL:6 * BL])
            c2 = trD[:, 6 * BL:7 * BL]
            dadd(trD[:, 0:BL], z[:, o0:o0 + BL], z[:, o0 + BL:o0 + 2 * BL])
            c3 = z[:, o0 + 2 * BL:o0 + 3 * BL]
            vP2 = small.tile([P, 256], fp32, tag="vP2")
            padd(vP2[:], vPa[:], c2)
            # merge on DVE: v_u = fold56 + c3 + vP2
            dadd(trD[:, BL:2 * BL], trD[:, 0:BL], c3)
            v_u = small.tile([P, 256], fp32, tag="vu")
            dadd(v_u[:], trD[:, BL:2 * BL], vP2[:])
            st["v_u"] = v_u

            if it == 2:
                nc.vector.reduce_sum(
                    S[:], e[:].rearrange("p (n o) -> p o n", n=NIN),
                    axis=AX.X)

        def squash(st, k):
            # ---- squash: out = v_u * sqrt(w2) / (S^2 + w2) ----
            v_u, S = st["v_u"], st["S"]
            sq = small.tile([P, 256], fp32, tag="sq")
            nc.vector.tensor_mul(sq[:], v_u[:], v_u[:])
            w2 = small.tile([P, O], fp32, tag="w2")
            nc.vector.reduce_sum(
                w2[:], sq[:].rearrange("p (l o) -> p o l", l=L), axis=AX.X)
            nc.vector.tensor_scalar_max(w2[:], w2[:], 1e-24)
            lg = small.tile([P, O], fp32, tag="lg")
            nc.scalar.activation(lg[:], w2[:], AF.Ln)
            sw = small.tile([P, O], fp32, tag="sw")
            nc.scalar.activation(sw[:], lg[:], AF.Exp, scale=0.5)
            den = small.tile([P, O], fp32, tag="den")
            nc.vector.tensor_mul(den[:], S[:], S[:])
            nc.vector.tensor_add(den[:], den[:], w2[:])
            rden = small.tile([P, O], fp32, tag="rn")
            nc.vector.reciprocal(rden[:], den[:])
            fac = small.tile([P, O], fp32, tag="fac")
            nc.vector.tensor_mul(fac[:], sw[:], rden[:])
            # vfin [p, (o,l)] = v_u viewed (o,l) * bcast_l(fac)
            vfin = small.tile([P, 256], fp32, tag="vfin")
            nc.vector.tensor_mul(
                vfin[:].rearrange("p (o l) -> p o l", o=O),
                v_u[:].rearrange("p (l o) -> p o l", l=L),
                fac[:].unsqueeze(2).broadcast_to([P, O, L]))
            # transpose to channel-major and store
            for half in range(2):
                tp = tpsum.tile([128, 128], fp32, tag="tp")
                nc.tensor.transpose(tp[:], vfin[:, half * 128:(half + 1) * 128],
                                    ident[:])
                vT = small.tile([128, 128], fp32, tag="vT")
                nc.scalar.copy(vT[:], tp[:])
                nc.sync.dma_start(
                    out=out_d[half * 128:(half + 1) * 128,
                              4 * k:4 * k + CHUNK_ROWS, :],
                    in_=vT[:].rearrange("f (r w) -> f r w", r=CHUNK_ROWS))

        # Interleave the two chunks' routing iterations: chunk k's small
        # ACT chains (prep/exp) overlap the other chunk's DVE work. Chunk
        # 1's priors are issued under chunk 0's first iteration so its ACT
        # copies don't delay chunk 0's start. prep is split: the rn chain
        # (prep_a) is emitted right after each main, the vn multiply
        # (prep_b) only after the OTHER chunk's main so the DVE queue head
        # never parks on the ACT round-trip.
        st0 = priors(0)
        prep_a(st0)
        prep_b(st0)
        main(st0, 0)
        prep_a(st0)
        st1 = priors(1)
        prep_a(st1)
        prep_b(st1)
        main(st1, 0)
        prep_a(st1)
        prep_b(st0)
        main(st0, 1)
        prep_a(st0)
        prep_b(st1)
        main(st1, 1)
        prep_a(st1)
        prep_b(st0)
        main(st0, 2)
        squash(st0, 0)
        prep_b(st1)
        main(st1, 2)
        squash(st1, 1)
    nc.compile()
    return nc


_NC_CACHE = {}


def _get_nc():
    if "nc" not in _NC_CACHE:
        _NC_CACHE["nc"] = _build_bass()
    return _NC_CACHE["nc"]


def _shard_inputs(x, weight):
    # wr[m, (t, l, o)] = weight[o, l, m, i, j], t = i*3+j
    wr = np.ascontiguousarray(
        weight.transpose(2, 3, 4, 1, 0).reshape(M, NTAP * 256)
        .astype(np.float32))
    in_maps = []
    for core in range(NCORES):
        b = core // 4
        oh0 = (core % 4) * ROWS_PER_CORE
        xs = np.zeros((CIN, 10, 34), np.float32)
        lo, hi = oh0 - 1, oh0 + 9
        vlo, vhi = max(lo, 0), min(hi, H)
        xs[:, vlo - lo:vhi - lo, 1:33] = x[b, :, vlo:vhi, :]
        # [c=(g,m), h, w34] -> [m, g, h, w34]
        xs_m = np.ascontiguousarray(
            xs.reshape(G, M, 10, 34).transpose(1, 0, 2, 3))
        in_maps.append({"xs": xs_m, "xs2": xs, "wgt": wr})
    return in_maps


def _gather_output(results):
    out = np.zeros((B, COUT, H, W), np.float32)
    for core in range(NCORES):
        b = core // 4
        oh0 = (core % 4) * ROWS_PER_CORE
        out[b, :, oh0:oh0 + ROWS_PER_CORE, :] = results[core]["out"]
    return out


def kernel(x: np.ndarray, weight: np.ndarray) -> np.ndarray:
    from concourse.bass_utils import run_bass_kernel_spmd

    x = np.asarray(x, np.float32)
    weight = np.asarray(weight, np.float32)
    res = run_bass_kernel_spmd(_get_nc(), _shard_inputs(x, weight),
                               list(range(NCORES)))
    return _gather_output(res.results)


# revision 45
# speedup vs baseline: 1.0018x; 1.0018x over previous
"""CapsuleConv2d (k-means routing, 3 iters) Trainium2 Bass kernel.

Problem (hardcoded): x [2,128,32,32] f32, weight [16,16,16,3,3] f32
(w[o,l,m,i,j]), stride 1, pad 1, G=8 groups of M=16 in-channels,
N_in = G*KH*KW = 72 votes, O=16 out-capsules of L=16.
Output [2, 256, 32, 32] f32.

Sharding: data-parallel over (b, oh): 64 rows -> 8 cores x 8 rows.
Each core processes 2 chunks of 128 positions (4 oh-rows x 32 ow).

Per-chunk pipeline (single NeuronCore):
  PE:  priors u[p,(n,l,o)] via 72 fp32r matmuls [K=16(m), M=128(p),
       N=256(l,o)] + 9 accumulating K=128 matmuls for v0 = sum_n u.
  ACT: PSUM->SBUF u drains (cast bf16), exp, rsqrt/sqrt via exp/ln
       (single act-table set: Exp/Ln/Copy/Identity pinned to one set).
  Routing elementwise work in layout [p partitions, (n, l, o) free] is
  SPLIT by n between Pool (n 0:16, tensor_add/mul at 0.42 Q7 efficiency
  ~2.0ns/elem) and DVE (n 16:72, bf16 2x mode 0.52ns/elem), per iter:
       z1 = u*bcast(vn) -> l-tree -> logits(bf16) -> e = exp (ACT, split
       3 ways so each engine's z2 gates only on its own logits)
       z2 = u*bcast(e) -> n-fold per engine -> v_u merge (Pool absorbs
       the DVE fold carry so the merge is 2 DVE adds)
  prep is split: rn chain right after each main, the vn multiply only
  after the OTHER chunk's main (DVE queue head never parks on ACT).
  final: squash fused with softmax denom: out = v_u*||v_u||/(S^2+||v_u||^2),
       PE-transpose [p,(o,l)] -> [(o,l),p], channel-major output DMA.
"""
from contextlib import ExitStack

import numpy as np

B, CIN, H, W = 2, 128, 32, 32
G, M, O, L = 8, 16, 16, 16
NTAP, NIN = 9, 72
COUT = O * L
NCORES = 8
ROWS_PER_CORE = 8  # (b, oh) rows per core
CHUNK_ROWS = 4
NCHUNK = ROWS_PER_CORE // CHUNK_ROWS
P = 128
NPOOL = 16               # n-blocks routed on the Pool engine (n 0:16)
NDVE = NIN - NPOOL       # 56, on DVE (n 16:72)
NA = 20                  # DVE section A (n 16:36; with Pool: exp half 1)
NB = NDVE - NA           # DVE section B (n 36:72, 36 blocks; exp half 2)
BL = 256                 # one n-block = L*O elems


def _build_bass():
    import concourse.tile as tile
    from concourse import bacc, masks, mybir

    # The act-table pass greedily picks the first set containing each
    # function, ping-ponging exp_and_others <-> natural_log (2.7us/load).
    # Strip Exp/Ln from every set except the combined one so all our ACT
    # work (Exp, Ln, Copy, Identity) lives in a single table set.
    if not getattr(bacc, "_capsule_act_tables_patched", False):
        _orig_gat = bacc.get_activation_tables

        def _gat(arch):
            t = dict(_orig_gat(arch))
            for name, fns in t.items():
                if name != "natural_log_exp_and_others":
                    t[name] = {f for f in fns
                               if f.name not in ("Exp", "Ln", "Copy",
                                                 "Identity")}
            return t

        bacc.get_activation_tables = _gat
        bacc._capsule_act_tables_patched = True

    fp32 = mybir.dt.float32
    f32r = mybir.dt.float32r
    bf16 = mybir.dt.bfloat16
    AX = mybir.AxisListType
    AF = mybir.ActivationFunctionType
    ALU = mybir.AluOpType

    nc = bacc.Bacc("TRN2", target_bir_lowering=False, debug=False)
    # host-pretransposed, pre-padded slab: xs[m, g, h(10), w(34)]
    xs_d = nc.declare_dram_parameter("xs", [M, G, 10, 34], f32r,
                                     isOutput=False)
    # same slab, channel-major: xs2[c=(g,m), h(10), w(34)]
    xs2_d = nc.declare_dram_parameter("xs2", [CIN, 10, 34], f32r,
                                      isOutput=False)
    # host-pretransposed weights: wr[m, (tap, l, o)] = w[o, l, m, i, j]
    w_d = nc.declare_dram_parameter("wgt", [M, NTAP * 256], fp32, isOutput=False)
    out_d = nc.declare_dram_parameter("out", [COUT, ROWS_PER_CORE, W], fp32,
                                      isOutput=True)

    with tile.TileContext(nc) as tc, ExitStack() as ctx:
        const_pool = ctx.enter_context(tc.tile_pool(name="const", bufs=1))
        upool = ctx.enter_context(tc.tile_pool(name="u", bufs=2))
        zpool = ctx.enter_context(tc.tile_pool(name="z", bufs=1))
        small = ctx.enter_context(tc.tile_pool(name="small", bufs=2))
        psum = ctx.enter_context(tc.tile_pool(name="ps", bufs=4, space="PSUM"))
        tpsum = ctx.enter_context(tc.tile_pool(name="tps", bufs=2, space="PSUM"))
        vpsum = ctx.enter_context(tc.tile_pool(name="vps", bufs=2, space="PSUM"))

        def dadd(out, a, b):
            nc.vector.tensor_add(out, a, b)

        def padd(out, a, b):
            nc.gpsimd.tensor_add(out, a, b)

        def pmul(out, a, b):
            nc.gpsimd.tensor_mul(out, a, b)

        # ---- constants (once per core) ----
        # x slab first: it gates the patch pipeline (weights only gate the
        # matmuls, which start later anyway)
        slab_f = const_pool.tile([M, G * 10 * 34], f32r)
        nc.sync.dma_start(out=slab_f[:],
                          in_=xs_d[:].rearrange("m g h w -> m (g h w)"))
        slab = slab_f[:].rearrange("m (g h w) -> m g h w", g=G, h=10)

        # channel-major slab for the K=128 v0 matmuls; f32r so the windowed
        # patch2 copies go over plain (cast-free) SP-queue DMAs
        slab2_f = const_pool.tile([CIN, 10 * 34], f32r)
        nc.sync.dma_start(out=slab2_f[:],
                          in_=xs2_d[:].rearrange("c h w -> c (h w)"))
        slab2 = slab2_f[:].rearrange("c (h w) -> c h w", h=10)

        # weights replicated over g: wr2[(g,m), (t,l,o)]; wr_r = g=0 slice.
        wr2_f = const_pool.tile([CIN, NTAP * 256], fp32)
        for g in range(G):
            eng = nc.sync if g % 2 == 0 else nc.scalar
            eng.dma_start(out=wr2_f[g * M:(g + 1) * M, :], in_=w_d[:])
        wr2 = const_pool.tile([CIN, NTAP * 256], f32r)
        nc.vector.tensor_copy(wr2[:], wr2_f[:])
        wr_r = wr2[0:M, :]

        ident = const_pool.tile([128, 128], fp32)
        masks.make_identity(nc, ident[:])

        # PE warm-up: ~4us of back-to-back dummy matmuls during the initial
        # DMA wait releases the HAM clock throttle before the real matmuls.
        warm = const_pool.tile([128, 64], bf16)
        nc.vector.memset(warm[:], 0.0)
        wps = tpsum.tile([64, 64], fp32, tag="tp")
        for _ in range(40):
            nc.tensor.matmul(wps[:], warm[:, 0:64], warm[:], start=True,
                             stop=True, tile_position=(0, 0))

        ppool = ctx.enter_context(tc.tile_pool(name="patch", bufs=4))
        p2pool = ctx.enter_context(tc.tile_pool(name="patch2", bufs=4))

        def priors(k):
            # ---- priors on PE (fp32r: full-rate, near-fp32 precision) ----
            # Chunk 0 (nothing to overlap with): Pool does patch2 + v0 copy
            # + the first two tap patches (everything its own z1P needs),
            # then goes idle so routing can start; ACT does the remaining
            # patches and all drains n>=16; DVE drains n 0:16 (Pool's
            # range) so z1P unblocks early. Chunk 1 (under chunk-0
            # routing): patch2 rides the idle SP DMA queue, patches and
            # drains go to ACT.
            u_sb = upool.tile([P, NIN * 256], bf16, tag="u")
            v0_ps = vpsum.tile([P, 256], fp32, tag="v0ps")
            v_u = small.tile([P, 256], fp32, tag="vu")
            # all v0 matmuls first: v0 completes early so the first prep
            # chain overlaps the u fill instead of following it
            for t in range(NTAP):
                i, j = divmod(t, 3)
                # channel-major window for the K=128 v0 matmul on the idle
                # SP DMA queue (3-dim AP fits the DMA cap)
                patch2 = p2pool.tile([CIN, P], f32r, tag="patch2")
                nc.sync.dma_start(
                    out=patch2[:].rearrange("c (h w) -> c h w",
                                            h=CHUNK_ROWS),
                    in_=slab2[:, 4 * k + i: 4 * k + i + CHUNK_ROWS,
                              j:j + W])
                nc.tensor.matmul(v0_ps[:], patch2[:],
                                 wr2[:, t * 256:(t + 1) * 256],
                                 start=(t == 0), stop=(t == NTAP - 1),
                                 tile_position=(0, 0), skip_group_check=True)
            nc.scalar.copy(v_u[:], v0_ps[:])
            for t in range(NTAP):
                i, j = divmod(t, 3)
                # contiguous tap window: patch[m, (g, poh, ow)]
                patch = ppool.tile([M, G * P], f32r, tag="patch")
                # chunk-0 fill is patch-cadence-bound: alternate Pool/ACT
                peng = (nc.gpsimd.tensor_copy if (k == 0 and t % 2 == 0)
                        else nc.scalar.copy)
                peng(
                    patch[:].rearrange("m (g h w) -> m g h w", g=G,
                                       h=CHUNK_ROWS),
                    slab[:, :, 4 * k + i: 4 * k + i + CHUNK_ROWS, j:j + W])
                rhs = wr_r[:, t * 256:(t + 1) * 256]
                for g in range(G):
                    n = t * G + g
                    lhsT = patch[:, g * P:(g + 1) * P]
                    if n % 2 == 0:
                        ups = psum.tile([P, 512], fp32, tag="ups")
                    nc.tensor.matmul(ups[:, (n % 2) * 256:(n % 2 + 1) * 256],
                                     lhsT, rhs, start=True, stop=True,
                                     tile_position=(0, 0))
                    if n % 2 == 1:
                        # drain two adjacent u blocks with one copy; during
                        # the chunk-0 fill DVE is idle, so share the load
                        dst = u_sb[:, (n - 1) * 256:(n + 1) * 256]
                        if k == 0 and (n // 2) % 2 == 0:
                            nc.vector.tensor_copy(dst, ups[:])
                        else:
                            nc.scalar.copy(dst, ups[:])
            S = small.tile([P, O], fp32, tag="S")
            return {"u_sb": u_sb, "v_u": v_u, "S": S, "vn": None}

        def prep_a(st, hi=True):
            # rn = rsqrt(max(sum_l v_u^2, eps)); rsqrt = exp(-ln/2)
            # hi=False for the post-priors preps: a high-priority Ln there
            # jumps to the ACT queue front and head-blocks the fill drains
            # behind it while waiting on w2
            v_u = st["v_u"]
            sq = small.tile([P, 256], fp32, tag="sq")
            nc.vector.tensor_mul(sq[:], v_u[:], v_u[:])
            w2 = small.tile([P, O], fp32, tag="w2")
            nc.vector.reduce_sum(
                w2[:], sq[:].rearrange("p (l o) -> p o l", l=L), axis=AX.X)
            nc.vector.tensor_scalar_max(w2[:], w2[:], 1e-24)
            lg = small.tile([P, O], fp32, tag="lg")
            rn = small.tile([P, O], fp32, tag="rn")
            if hi:
                with tc.high_priority():
                    nc.scalar.activation(lg[:], w2[:], AF.Ln)
                    nc.scalar.activation(rn[:], lg[:], AF.Exp, scale=-0.5)
            else:
                nc.scalar.activation(lg[:], w2[:], AF.Ln)
                nc.scalar.activation(rn[:], lg[:], AF.Exp, scale=-0.5)
            st["rn"] = rn

        def prep_b(st):
            # vn = v_u * bcast(rn); emitted AFTER the other chunk's main so
            # the DVE queue head never blocks on the ACT Ln/Exp round-trip
            v_u, rn = st["v_u"], st["rn"]
            vn = small.tile([P, 256], bf16, tag="vn")
            nc.vector.tensor_mul(
                vn[:].rearrange("p (l o) -> p l o", l=L),
                v_u[:].rearrange("p (l o) -> p l o", l=L),
                rn[:].unsqueeze(1).broadcast_to([P, L, O]))
            st["vn"] = vn

        def main(st, it):
            u_sb, vn, S = st["u_sb"], st["vn"], st["S"]
            # one tile: z region [0:18432] + tree scratch
            # scratch layout (bf16 elems):
            #   trD  [18432:25600]  7168  DVE l-tree + n-fold ping-pong
            #   trP  [25600:28160]  2560  Pool l-tree + n-fold ping-pong
            zt = zpool.tile([P, NIN * BL + 9728], bf16, tag="z")
            z = zt[:, 0:NIN * BL]
            trD = zt[:, NIN * BL:NIN * BL + 7168]
            trP = zt[:, NIN * BL + 7168:NIN * BL + 9728]

            # bf16 logits: |logits| <= ~5.1 so bf16 rounding (<=0.02 abs)
            # perturbs exp by <2%; buys 2x mode for the final l-tree adds
            logits = small.tile([P, NIN * O], bf16, tag="logits")
            e = small.tile([P, NIN * O], bf16, tag="e")

            vnb = lambda nn: (vn[:].rearrange("p (l o) -> p l o", l=L)
                              .unsqueeze(1).broadcast_to([P, nn, L, O]))

            def zview(ap, nn, lw=L):
                return ap.rearrange("p (n l o) -> p n l o", n=nn, l=lw)

            def ltree(eng_add, zsec, nn, tr, lgsec):
                # zsec: z-slice [p, nn*256]; tr: scratch >= nn*128
                zv = zview(zsec, nn)
                t1 = tr[:, 0:nn * 128].rearrange(
                    "p (n l o) -> p n l o", n=nn, l=8)
                eng_add(t1, zv[:, :, 0:8, :], zv[:, :, 8:16, :])
                t2 = zview(zsec[:, 0:nn * 64], nn, 4)
                eng_add(t2, t1[:, :, 0:4, :], t1[:, :, 4:8, :])
                t3 = tr[:, 0:nn * 32].rearrange(
                    "p (n l o) -> p n l o", n=nn, l=2)
                eng_add(t3, t2[:, :, 0:2, :], t2[:, :, 2:4, :])
                eng_add(lgsec.rearrange("p (n o) -> p n o", n=nn),
                        t3[:, :, 0, :], t3[:, :, 1, :])

            # Section layout: Pool owns n 0:16 (zP), DVE owns n 16:36 (zA)
            # and 36:72 (zB). Three exp splits so each engine's z2 gates
            # only on its own logits (no cross-engine phase lag).
            zP = z[:, 0:NPOOL * BL]
            zA = z[:, NPOOL * BL:(NPOOL + NA) * BL]
            zB = z[:, (NPOOL + NA) * BL:NIN * BL]
            uP = zview(u_sb[:, 0:NPOOL * BL], NPOOL)
            uA = zview(u_sb[:, NPOOL * BL:(NPOOL + NA) * BL], NA)
            uB = zview(u_sb[:, (NPOOL + NA) * BL:NIN * BL], NB)

            # --- z1 = u * bcast_n(vn), l-tree -> logits ---
            pmul(zview(zP, NPOOL), uP, vnb(NPOOL))
            ltree(padd, zP, NPOOL, trP, logits[:, 0:NPOOL * O])
            nc.vector.tensor_mul(zview(zA, NA), uA, vnb(NA))
            ltree(dadd, zA, NA, trD, logits[:, NPOOL * O:(NPOOL + NA) * O])
            # exp order by expected logits readiness: A (~5us), P (~14), B
            with tc.high_priority():
                nc.scalar.activation(
                    e[:, NPOOL * O:(NPOOL + NA) * O],
                    logits[:, NPOOL * O:(NPOOL + NA) * O], AF.Exp)
            nc.vector.tensor_mul(zview(zB, NB), uB, vnb(NB))
            ltree(dadd, zB, NB, trD[:, NA * 128:NA * 128 + NB * 128],
                  logits[:, (NPOOL + NA) * O:NIN * O])
            with tc.high_priority():
                nc.scalar.activation(e[:, 0:NPOOL * O],
                                     logits[:, 0:NPOOL * O], AF.Exp)
                nc.scalar.activation(e[:, (NPOOL + NA) * O:NIN * O],
                                     logits[:, (NPOOL + NA) * O:NIN * O],
                                     AF.Exp)

            def ebc(lo, nn):
                return (e[:, lo * O:(lo + nn) * O]
                        .rearrange("p (n o) -> p n o", n=nn)
                        .unsqueeze(2).broadcast_to([P, nn, L, O]))

            # --- z2 = u * bcast_l(e) ---
            pmul(zview(zP, NPOOL), uP, ebc(0, NPOOL))
            nc.vector.tensor_mul(zview(zA, NA), uA, ebc(NPOOL, NA))
            nc.vector.tensor_mul(zview(zB, NB), uB, ebc(NPOOL + NA, NB))

            # --- n-fold: Pool over its 16 blocks, DVE over its 56 ---
            # Pool: 16 -> 8 -> 4 -> 2 -> 1 (clean), then absorb the DVE
            # carry c2 (Pool leads in phase; saves DVE merge adds)
            padd(trP[:, 0:8 * BL], zP[:, 0:8 * BL], zP[:, 8 * BL:16 * BL])
            padd(zP[:, 0:4 * BL], trP[:, 0:4 * BL], trP[:, 4 * BL:8 * BL])
            padd(trP[:, 8 * BL:10 * BL], zP[:, 0:2 * BL],
                 zP[:, 2 * BL:4 * BL])
            vPa = small.tile([P, 256], fp32, tag="vPa")
            padd(vPa[:], trP[:, 8 * BL:9 * BL], trP[:, 9 * BL:10 * BL])

            # DVE: 56 -> 28 -> 14 -> 7 -> 3(+c2) -> 1(+c3); blocks start
            # at offset 16*BL (zD = z[16BL:72BL])
            o0 = NPOOL * BL
            dadd(trD[:, 0:28 * BL], z[:, o0:o0 + 28 * BL],
                 z[:, o0 + 28 * BL:o0 + 56 * BL])
            dadd(z[:, o0:o0 + 14 * BL], trD[:, 0:14 * BL],
                 trD[:, 14 * BL:28 * BL])
            dadd(trD[:, 0:7 * BL], z[:, o0:o0 + 7 * BL],
                 z[:, o0 + 7 * BL:o0 + 14 * BL])
            dadd(z[:, o0:o0 + 3 * BL], trD[:, 0:3 * BL], trD[:, 3 * BL:6 * BL])
            c2 = trD[:, 6 * BL:7 * BL]
            dadd(trD[:, 0:BL], z[:, o0:o0 + BL], z[:, o0 + BL:o0 + 2 * BL])
            c3 = z[:, o0 + 2 * BL:o0 + 3 * BL]
            vP2 = small.tile([P, 256], fp32, tag="vP2")
            padd(vP2[:], vPa[:], c2)
            # merge on DVE: v_u = fold56 + c3 + vP2
            dadd(trD[:, BL:2 * BL], trD[:, 0:BL], c3)
            v_u = small.tile([P, 256], fp32, tag="vu")
            dadd(v_u[:], trD[:, BL:2 * BL], vP2[:])
            st["v_u"] = v_u

            if it == 2:
                nc.vector.reduce_sum(
                    S[:], e[:].rearrange("p (n o) -> p o n", n=NIN),
                    axis=AX.X)

        def squash(st, k):
            # ---- squash: out = v_u * sqrt(w2) / (S^2 + w2) ----
            v_u, S = st["v_u"], st["S"]
            sq = small.tile([P, 256], fp32, tag="sq")
            nc.vector.tensor_mul(sq[:], v_u[:], v_u[:])
            w2 = small.tile([P, O], fp32, tag="w2")
            nc.vector.reduce_sum(
                w2[:], sq[:].rearrange("p (l o) -> p o l", l=L), axis=AX.X)
            nc.vector.tensor_scalar_max(w2[:], w2[:], 1e-24)
            lg = small.tile([P, O], fp32, tag="lg")
            nc.scalar.activation(lg[:], w2[:], AF.Ln)
            sw = small.tile([P, O], fp32, tag="sw")
            nc.scalar.activation(sw[:], lg[:], AF.Exp, scale=0.5)
            den = small.tile([P, O], fp32, tag="den")
            nc.vector.tensor_mul(den[:], S[:], S[:])
            nc.vector.tensor_add(den[:], den[:], w2[:])
            rden = small.tile([P, O], fp32, tag="rn")
            nc.vector.reciprocal(rden[:], den[:])
            fac = small.tile([P, O], fp32, tag="fac")
            nc.vector.tensor_mul(fac[:], sw[:], rden[:])
            # vfin [p, (o,l)] = v_u viewed (o,l) * bcast_l(fac)
            vfin = small.tile([P, 256], fp32, tag="vfin")
            nc.vector.tensor_mul(
                vfin[:].rearrange("p (o l) -> p o l", o=O),
                v_u[:].rearrange("p (l o) -> p o l", l=L),
                fac[:].unsqueeze(2).broadcast_to([P, O, L]))
            # transpose to channel-major and store
            for half in range(2):
                tp = tpsum.tile([128, 128], fp32, tag="tp")
                nc.tensor.transpose(tp[:], vfin[:, half * 128:(half + 1) * 128],
                                    ident[:])
                vT = small.tile([128, 128], fp32, tag="vT")
                nc.scalar.copy(vT[:], tp[:])
                nc.sync.dma_start(
                    out=out_d[half * 128:(half + 1) * 128,
                              4 * k:4 * k + CHUNK_ROWS, :],
                    in_=vT[:].rearrange("f (r w) -> f r w", r=CHUNK_ROWS))

        # Interleave the two chunks' routing iterations: chunk k's small
        # ACT chains (prep/exp) overlap the other chunk's DVE work. Chunk
        # 1's priors are issued under chunk 0's first iteration so its ACT
        # copies don't delay chunk 0's start. prep is split: the rn chain
        # (prep_a) is emitted right after each main, the vn multiply
        # (prep_b) only after the OTHER chunk's main so the DVE queue head
        # never parks on the ACT round-trip.
        st0 = priors(0)
        prep_a(st0)
        prep_b(st0)
        main(st0, 0)
        prep_a(st0)
        st1 = priors(1)
        prep_a(st1)
        prep_b(st1)
        main(st1, 0)
        prep_a(st1)
        prep_b(st0)
        main(st0, 1)
        prep_a(st0)
        prep_b(st1)
        main(st1, 1)
        prep_a(st1)
        prep_b(st0)
        main(st0, 2)
        squash(st0, 0)
        prep_b(st1)
        main(st1, 2)
        squash(st1, 1)
    nc.compile()
    return nc


_NC_CACHE = {}


def _get_nc():
    if "nc" not in _NC_CACHE:
        _NC_CACHE["nc"] = _build_bass()
    return _NC_CACHE["nc"]


def _shard_inputs(x, weight):
    # wr[m, (t, l, o)] = weight[o, l, m, i, j], t = i*3+j
    wr = np.ascontiguousarray(
        weight.transpose(2, 3, 4, 1, 0).reshape(M, NTAP * 256)
        .astype(np.float32))
    in_maps = []
    for core in range(NCORES):
        b = core // 4
        oh0 = (core % 4) * ROWS_PER_CORE
        xs = np.zeros((CIN, 10, 34), np.float32)
        lo, hi = oh0 - 1, oh0 + 9
        vlo, vhi = max(lo, 0), min(hi, H)
        xs[:, vlo - lo:vhi - lo, 1:33] = x[b, :, vlo:vhi, :]
        # [c=(g,m), h, w34] -> [m, g, h, w34]
        xs_m = np.ascontiguousarray(
            xs.reshape(G, M, 10, 34).transpose(1, 0, 2, 3))
        in_maps.append({"xs": xs_m, "xs2": xs, "wgt": wr})
    return in_maps


def _gather_output(results):
    out = np.zeros((B, COUT, H, W), np.float32)
    for core in range(NCORES):
        b = core // 4
        oh0 = (core % 4) * ROWS_PER_CORE
        out[b, :, oh0:oh0 + ROWS_PER_CORE, :] = results[core]["out"]
    return out


def kernel(x: np.ndarray, weight: np.ndarray) -> np.ndarray:
    from concourse.bass_utils import run_bass_kernel_spmd

    x = np.asarray(x, np.float32)
    weight = np.asarray(weight, np.float32)
    res = run_bass_kernel_spmd(_get_nc(), _shard_inputs(x, weight),
                               list(range(NCORES)))
    return _gather_output(res.results)
